# revision 35
# baseline (speedup 1.0000x reference)
"""Trainium2 Bass kernel for nn_DiscreteMMSE — f32r pipeline, PE-side numerator.

Reference computation (per batch b):
    proj[n,t] = data[b,n,:] @ W[:,t]
    logp      = -0.5*(targets - proj)^2 + const
    csum      = cumsum_n(logp);  alpha = softmax_t(csum[n-1])
    pred[n]   = sum_t alpha[n-1,t] * proj[n,t]   (n>=1)
    pred[0]   = data[b,0,:] @ W.mean(axis=1)

Design (vs the 168us baseline whose DVE ran min+stt at 2384ns/chunk):
  * err[n,t] = y - x.w via ONE augmented f32r matmul (lhsT=[data^T;y],
    rhs=[-W;1]) -> err_ps. ACT squares it (err2, f32r SBUF); PE csum via
    POSITIVE 0.5*L_strict matmul -> csum_ps; DVE min -> ACT exp(scale=-1,
    bias=-min) -> expw f16. Row 0 degenerates to uniform weights == the
    reference's prior-mean pred0 (no special case).
  * NEW numerator path: instead of recomputing err (late matmul) and a
    full-width DVE scalar_tensor_tensor (1192ns), note
        sum_t expw*proj = x . (W_chunk @ expw^T)      (w~ = W @ expw^T)
        sum_t expw      = ones @ expw^T               (denominator)
    One SBUF->SBUF dma_start_transpose per chunk turns expw [128,1024]
    into 8 stacked [128t,128n] f16 tiles (idle DMA hw, issued from idle
    SP). PE contracts them with host-packed [W^T|1] f16 tiles into
    wt_ps[n,65] (8 x 65-col f16 matmuls, ~520 PE cycles). DVE then does a
    64-el stt (x16 . wt) for the numerator and a 1-el copy for the
    denominator column. DVE/chunk: 2384 -> ~1650ns. ACT drops the exp
    accum_out (denominator now free from the ones row): 2263 -> ~2080ns,
    the new steady-state bottleneck (91%+ busy). PE: ~2570 cols/chunk,
    fully ramped to 2.4GHz via startup warmup dummies.
  * PSUM: err 1 slot (2 banks) + csum 3 slots (6 banks) = all 8 banks;
    the wt accumulators alias the first 65 cols of csum slot (k+2)%3,
    dead between exp(k+2) and csum(k+5).
  * ACT runs exp(k) TWO cycles after min(k) (act order [exp(k), sq(k+3)])
    so the sq->csum->min->exp chain (~2.5us) never stalls ACT; the wt
    matmuls trail by WLAG=4 chunks to hide the ~3.1us transpose-DMA chain.
  * per-batch negMg + delta on idle DVE mid-loop; ONE 64-col exp at the
    end replaces 16 per-batch cq activations.
  * tail: cq = exp(M_q - M_b);
    pred = (sum_q cq*numer_q) / (sum_q cq*denom_q)  -- no targets needed.

Raw bass with explicit semaphores via a small planner (one wait_ge per
dependency, elided when implied by program order or earlier waits).

Sharded batch-parallel over 8 cores: 16 batches/core, W replicated.
kernel() host-packs ops/W^T/x16 per core (pure data marshalling; all
compute runs on device).
"""

from contextlib import ExitStack

import numpy as np

import concourse.bass as bass
from concourse import mybir
from concourse.bass_utils import run_bass_kernel_spmd

B, N, D, T = 128, 128, 64, 4096
NCORES = 8
BS = B // NCORES          # batches per core
CW = 1024                 # task-axis chunk width (2 PSUM banks fp32)
NQ = T // CW              # chunks per batch
MM = 512                  # one PSUM bank of fp32
NK = BS * NQ              # total chunk count
NH = NK * 2               # total half (512) count
NT = CW // 128            # 128-wide t-tiles per chunk (transpose grain)
NG = T // 128             # global t-tile count (W^T tiles)

F32 = mybir.dt.float32
F32R = mybir.dt.float32r
F16 = mybir.dt.float16
AX = mybir.AxisListType.X
OP = mybir.AluOpType
AF = mybir.ActivationFunctionType

import os
NCSUM = 3                 # csum PSUM slots (3 x 2 banks)
SELF_WAITS = os.environ.get("SELF_WAITS", "1") == "1"
WLAG = int(os.environ.get("WLAG", "4"))    # wt lag behind exp-chunk index
NE2 = 3                   # err2 chunk buffers
NEXPW = 5                 # expw chunk buffers
NTR = 4                   # expwT chunk buffers
NSCR = 4                  # stt dummy-output rotation (avoid DVE WAW overlap)
SELF_LAG = int(os.environ.get("SELF_LAG", "2"))


class Planner:
    """Per-engine step lists with resolved single-sem wait thresholds."""

    def __init__(self):
        self.steps = {"PE": [], "ACT": [], "DVE": [], "POOL": [], "SP": []}
        self.counts = {}
        self.waited = {e: {} for e in self.steps}

    self_waits = True
    self_lag = 1

    def step(self, eng, emit, waits=(), inc=None, fuse=True):
        waits = list(waits)
        if self.self_waits and eng in ("DVE", "POOL"):
            prev = self.counts.get(eng.lower(), 0) - (self.self_lag - 1)
            if prev > 0:
                waits.insert(0, (eng.lower(), prev))
        real = []
        for sem_name, thr in waits:
            if thr is None or thr <= 0:
                continue
            if self.waited[eng].get(sem_name, 0) >= thr:
                continue
            self.waited[eng][sem_name] = thr
            real.append((sem_name, thr))
        if inc is None:
            inc = (eng.lower(), 1)
        if inc is not False:
            self.counts.setdefault(inc[0], 0)
            self.counts[inc[0]] += inc[1]
        self.steps[eng].append(
            (emit, real, inc if inc is not False else None, fuse))
        return self.counts[inc[0]] if inc is not False else None


def build_nc():
    nc = bass.Bass("TRN2")
    ctx = ExitStack()
    ctx.enter_context(nc.allow_low_precision(reason="f32r/f16 pipeline"))

    # host-packed operands, laid out [lhsT_b0(128) | rhs(4096) | lhsT_rest]
    # so the first DMA (1152 cols) unblocks batch-0/chunk-0 work asap.
    OPS_W = BS * N + T
    ops_h = nc.dram_tensor("ops_p", [D + 1, OPS_W], F32,
                           kind="ExternalInput")
    wt16_h = nc.dram_tensor("wt16_p", [128, NG * (D + 1)], F16,
                            kind="ExternalInput")
    xn16_h = nc.dram_tensor("xn16_p", [N, BS * D], F16, kind="ExternalInput")
    out_h = nc.dram_tensor("out_s", [N, BS], F32, kind="ExternalOutput")
    # POSITIVE 0.5*strict-lower (transposed): csum_ps = -csum; the row-0
    # column of L is all-zero, so row 0 of the pipeline degenerates to
    # uniform weights == the reference's prior-mean pred0. No special case.
    l_h = nc.dram_tensor("lmat_p", [N, N], F32, kind="ExternalInput")

    def sb(name, shape, dt):
        return ctx.enter_context(nc.sbuf_tensor(name, shape, dt))

    def ps(name, shape, dt):
        return ctx.enter_context(nc.psum_tensor(name, shape, dt))

    l_sb = sb("l_sb", [N, N], F32R)
    ops = sb("ops", [D + 1, OPS_W], F32R)
    rhs = ops[:, N:N + T]                      # [-W ; ones]
    l_r = l_sb[:]

    def lhs_view(b):
        # batch 0 lives at cols 0:128; batches 1.. after rhs
        c0 = 0 if b == 0 else N + T + (b - 1) * N
        return ops[:, c0:c0 + N]
    wt16 = sb("wt16", [128, NG, D + 1], F16)   # [W^T | 1] t-tiles
    xn16 = sb("xn16", [N, BS, D], F16)         # data in [n, d] layout
    preds = sb("preds", [N, BS], F32)
    err2 = [sb(f"err2_{i}", [N, CW], F32R) for i in range(NE2)]
    expw = [sb(f"expw_{i}", [N, CW], F16) for i in range(NEXPW)]
    expwT = [sb(f"expwT_{i}", [128, NT, 128], F16) for i in range(NTR)]
    scr = [sb(f"scr_{i}", [N, D], F32) for i in range(NSCR)]
    negMq = sb("negMq", [N, NK], F32)
    dq_all = sb("dq_all", [N, NK], F32)
    nq_all = sb("nq_all", [N, NK], F32)
    cq_all = sb("cq_all", [N, NK], F32)
    cqd = sb("cqd", [N, NK], F32)
    prod = sb("prod", [N, NK], F32)
    negMg_t = sb("negMg_t", [N, BS], F32)
    Dall = sb("Dall", [N, BS], F32)
    rDall = sb("rDall", [N, BS], F32)
    Sraw = sb("Sraw", [N, BS], F32)

    # PSUM: err 1 slot (2 banks) + csum 3 slots (6 banks) = full 16KB.
    # ACT runs exp(k) TWO cycles after min(k) (act cycle = [exp(k),
    # sq(k+3)]), so the sq->csum->min->exp chain (~2.5us) spans two act
    # cycles and ACT never stalls on it. The wt accumulators alias the
    # first 65 cols of csum slot (k+2)%3, which is dead from exp(k+2)
    # until csum(k+5).
    err_ps = ps("err_ps", [N, CW], F32)
    csum_ps = [ps(f"csum_ps_{i}", [N, CW], F32) for i in range(NCSUM)]
    wt_view = [csum_ps[i][:, 0:D + 1] for i in range(NCSUM)]

    zcol = sb("zcol", [128, 1], F32)
    dum = sb("dum", [128, 256], F32R)

    P = Planner()
    P.self_waits = SELF_WAITS
    P.self_lag = SELF_LAG

    # ---------------- DMAs ----------------
    # all operands are f32/f16 (f32r is a bitcast view), so no casting
    # DMAs are needed: the critical startup pieces ride SP/HWDGE (fast
    # issue), the rest go on the pool/SWDGE queue in parallel.
    def dma_ops(eng, c0, c1):
        return lambda: eng.dma_start(out=ops[:, c0:c1],
                                     in_=ops_h[:, c0:c1])

    pool_dmas = [
        ("dr0", dma_ops(nc.gpsimd, 0, N + CW)),    # lhsT_b0 + rhs chunk 0
        ("dl", lambda: nc.gpsimd.dma_start(out=l_sb[:], in_=l_h[:])),
        ("dr1", dma_ops(nc.gpsimd, N + CW, N + 2 * CW)),
        ("dr2", dma_ops(nc.gpsimd, N + 2 * CW, N + 4 * CW)),
        ("dx1", dma_ops(nc.gpsimd, N + T, N + T + 5 * N)),  # lhsT b1-5
        ("dx2", dma_ops(nc.gpsimd, N + T + 5 * N, N + T + 10 * N)),
        ("dx3", dma_ops(nc.gpsimd, N + T + 10 * N, N + T + 15 * N)),
    ]
    for s, d in pool_dmas:
        P.step("POOL", d, inc=(s, 16))
    # f16 operands are non-casting; issue from the idle SP/HWDGE queue
    sp_dmas = [
        ("dwt", lambda: nc.sync.dma_start(
            out=wt16[:], in_=wt16_h[:].rearrange(
                "p (g c) -> p g c", c=D + 1))),
        ("dxn", lambda: nc.sync.dma_start(
            out=xn16[:], in_=xn16_h[:].rearrange(
                "p (b d) -> p b d", d=D))),
    ]
    for s, d in sp_dmas:
        P.step("SP", d, inc=(s, 16))
    e_zcol = P.step("DVE", lambda: nc.vector.memset(zcol[:], 0.0))

    # ---------------- main loop ----------------
    # chunk k in [0, NK): 1024 tasks. err pair -> sq -> csum -> min ->
    # exp -> dma-transpose -> wt matmuls -> stt/dqc. Steady state
    # (ACT-bound ~2080ns):
    #   PE  [csum(k), err pair(k+2), wt(k-WLAG)x8]   ~2570 cols
    #   ACT [sq(k+1), exp(k)]                        ~2080
    #   DVE [min(k), stt(k-WLAG), dqc(k-WLAG)]       ~1650
    #   SP  [tr(k)]; DMA 64 xbar tiles/chunk
    t_err, t_sq, t_csum, t_min, t_exp = {}, {}, {}, {}, {}
    t_tr, t_wt, t_stt, t_dqc = {}, {}, {}, {}

    def pe_err(j):
        k = j // 2
        b, q = divmod(k, NQ)
        c0 = q * CW + (j % 2) * MM
        dst = err_ps[:, (j % 2) * MM:(j % 2) * MM + MM]
        rq = "dr0" if q == 0 else ("dr1" if q == 1 else "dr2")
        xq = "dr0" if b == 0 else f"dx{min(3, 1 + (b - 1) // 5)}"
        w = [(xq, 16), (rq, 16)]
        if k >= 1:
            w.append(("act", t_sq[k - 1]))     # slot reused after sq(k-1)
        t_err[j] = P.step("PE", (lambda b=b, c0=c0, dst=dst: nc.tensor.matmul(
            dst, lhs_view(b), rhs[:, c0:c0 + MM],
            start=True, stop=True)), w)

    def act_sq(k):
        dst = err2[k % NE2]
        w = [("pe", t_err[2 * k + 1]), ("dve", e_zcol)]
        kf = k - NE2
        if kf >= 0:
            w.append(("pe", t_csum[kf]))
        t_sq[k] = P.step("ACT", (lambda dst=dst:
                                 nc.scalar.activation(
            out=dst[:], in_=err_ps[:], func=AF.Square, bias=zcol[:],
            scale=1.0)), w)

    def pe_csum(k):
        s = k % NCSUM
        w = [("act", t_sq[k])]
        if k < 2:
            w.append(("dl", 16))
        kf = k - NCSUM
        if kf >= 0:
            w.append(("act", t_exp[kf]))       # full-slot reuse
        ka = k - WLAG - 1                      # wt(ka) aliased cols 0:65
        if ka >= 0:
            w.append(("dve", t_dqc[ka]))

        def emit(s=s, k=k):
            e2 = err2[k % NE2]
            nc.tensor.matmul(csum_ps[s][:, 0:MM], l_r, e2[:, 0:MM],
                             start=True, stop=True)
            return nc.tensor.matmul(csum_ps[s][:, MM:2 * MM], l_r,
                                    e2[:, MM:2 * MM], start=True, stop=True)
        t_csum[k] = P.step("PE", emit, w, fuse=False)

    def dve_min(k):
        s = k % NCSUM
        t_min[k] = P.step("DVE", (lambda k=k, s=s: nc.vector.tensor_reduce(
            out=negMq[:, k:k + 1], in_=csum_ps[s][:], axis=AX, op=OP.min)),
            [("pe", t_csum[k])])

    def act_exp(k):
        s = k % NCSUM
        # expw-buffer reuse (tr(k-NEXPW) done) is implied: exp waits
        # min(k) <- csum(k) <- PE order after wt(k-WLAG-1) whose first
        # matmul waited dtr >= 16*(k-WLAG) = 16*(k-NEXPW+1).
        w = [("dve", t_min[k])]
        t_exp[k] = P.step("ACT", (lambda k=k, s=s: nc.scalar.activation(
            out=expw[k % NEXPW][:], in_=csum_ps[s][:], func=AF.Exp,
            bias=negMq[:, k:k + 1], scale=-1.0)), w)

    def sp_tr(k):
        w = [("act", t_exp[k])]
        kf = k - NTR
        if kf >= 0:
            w.append(("pe", t_wt[kf]))         # expwT slot reused after wt
        t_tr[k] = P.step("SP", (lambda k=k: nc.sync.dma_start_transpose(
            out=expwT[k % NTR][:], in_=expw[k % NEXPW][:])),
            w, inc=("dtr", 16))

    def pe_wt(k):
        s = (k + 2) % NCSUM
        b, q = divmod(k, NQ)
        ke = min(k + 2, NK - 1)                # alias slot dead after exp
        w = [("dtr", 16 * (k + 1)), ("dwt", 16), ("act", t_exp[ke])]
        if k >= NCSUM:
            w.append(("dve", t_dqc[k - NCSUM]))  # prior alias use drained

        def emit(s=s, k=k, q=q):
            last = None
            for g in range(NT):
                last = nc.tensor.matmul(
                    wt_view[s], expwT[k % NTR][:, g, :],
                    wt16[:, NT * q + g, :],
                    start=(g == 0), stop=(g == NT - 1))
            return last
        t_wt[k] = P.step("PE", emit, w, fuse=False)

    def dve_stt(k):
        s = (k + 2) % NCSUM
        b = k // NQ
        t_stt[k] = P.step("DVE", (lambda k=k, s=s, b=b:
                                  nc.vector.scalar_tensor_tensor(
            out=scr[k % NSCR][:], in0=wt_view[s][:, 0:D], scalar=1.0,
            in1=xn16[:, b, :], op0=OP.mult, op1=OP.mult,
            accum_out=nq_all[:, k:k + 1])),
            [("pe", t_wt[k]), ("dxn", 16)])

    def dve_dqc(k):
        s = (k + 2) % NCSUM
        t_dqc[k] = P.step("DVE", (lambda k=k, s=s: nc.vector.tensor_scalar(
            out=dq_all[:, k:k + 1], in0=wt_view[s][:, D:D + 1], scalar1=1.0,
            scalar2=None, op0=OP.mult)),
            [("pe", t_wt[k])])

    # PE p-state warmup: ~14 x 256-col f32r dummy matmuls (~5.5us of PE
    # busy from t~0.4us) keep the ramp alive until the first real err
    # matmuls, which then run at full 2.4GHz instead of 0.65/1.2.
    for _ in range(14):
        P.step("PE", lambda: nc.tensor.matmul(
            err_ps[:, 0:256], dum[:, 0:128], dum[:],
            start=True, stop=True), [], inc=False)

    # emission order per engine (greedy against emission-time deps)
    pe_q = [("err", 0), ("err", 1)]
    for k in range(NK):
        if 2 * k + 2 < NH:
            pe_q.append(("err", 2 * k + 2))
            pe_q.append(("err", 2 * k + 3))
        pe_q.append(("csum", k))
        if k - WLAG >= 0:
            pe_q.append(("wt", k - WLAG))
    for k in range(NK - WLAG, NK):
        pe_q.append(("wt", k))

    t_mg = {}
    t_dlt = {}

    def dve_mg(b):
        # per-batch negMg on idle DVE mid-loop; placed >=2 DVE ops after
        # min(4b+3) so the lag-2 self-wait proves it complete.
        t_mg[b] = P.step("DVE", (lambda b=b: nc.vector.tensor_reduce(
            out=negMg_t[:, b:b + 1],
            in_=negMq[:, b * NQ:(b + 1) * NQ], axis=AX, op=OP.min)), [])

    def dve_dlt(b):
        # cqd = negMq - negMg (broadcast); >=2 DVE ops after mg(b)
        t_dlt[b] = P.step("DVE", (lambda b=b: nc.vector.tensor_scalar(
            out=cqd[:, b * NQ:(b + 1) * NQ],
            in0=negMq[:, b * NQ:(b + 1) * NQ],
            scalar1=negMg_t[:, b:b + 1], scalar2=None,
            op0=OP.subtract)), [])

    # ACT cycle = [exp(k), sq(k+3)]: exp trails min(k) by two cycles, so
    # the sq->csum->min chain never stalls ACT.
    act_q = [("sq", 0), ("sq", 1), ("sq", 2)]
    dve_q = []
    sp_q = []
    for k in range(NK):
        act_q.append(("exp", k))
        if k + 3 < NK:
            act_q.append(("sq", k + 3))
        sp_q.append(("tr", k))
        dve_q.append(("min", k))
        if k - WLAG >= 0:
            dve_q.append(("stt", k - WLAG))
        if k - WLAG >= 0:
            dve_q.append(("dqc", k - WLAG))
        # lag-2 self-waits imply: at issue of op p, all ops <= p-2 are
        # complete. mg(b) sits >=2 after min(4b+3); dlt(b) one cycle later,
        # >=2 after mg(b). Placed last so dqc (alias chain) isn't delayed.
        if k % NQ == 0 and k > 0:
            dve_q.append(("mg", k // NQ - 1))
        if k % NQ == 1 and k > NQ:
            dve_q.append(("dlt", k // NQ - 1))
    flush = []
    for k in range(NK - WLAG, NK):
        flush.append(("stt", k))
        flush.append(("dqc", k))
    flush.insert(2, ("mg", BS - 1))
    flush.insert(5, ("dlt", BS - 1))
    dve_q.extend(flush)

    def deps_ready(item):
        kind, a = item
        if kind == "err":
            return a // 2 < 1 or (a // 2 - 1) in t_sq
        if kind == "sq":
            if (2 * a + 1) not in t_err:
                return False
            kf = a - NE2
            return kf < 0 or kf in t_csum
        if kind == "csum":
            if a not in t_sq:
                return False
            kf = a - NCSUM
            if kf >= 0 and kf not in t_exp:
                return False
            ka = a - WLAG - 1
            return ka < 0 or ka in t_dqc
        if kind == "min":
            return a in t_csum
        if kind == "exp":
            if a not in t_min:
                return False
            kf = a - NEXPW
            return kf < 0 or kf in t_tr
        if kind == "tr":
            if a not in t_exp:
                return False
            kf = a - NTR
            return kf < 0 or kf in t_wt
        if kind == "wt":
            if a not in t_tr:
                return False
            if min(a + 2, NK - 1) not in t_exp:
                return False
            kf = a - NCSUM
            return kf < 0 or kf in t_dqc
        if kind == "stt":
            return a in t_wt
        if kind == "dqc":
            return a in t_wt and a in t_stt
        if kind == "mg":
            return (a * NQ + 3) in t_min
        if kind == "dlt":
            return a in t_mg
        raise ValueError(kind)

    emitters = {"err": pe_err, "sq": act_sq, "csum": pe_csum, "min": dve_min,
                "exp": act_exp, "tr": sp_tr, "wt": pe_wt, "stt": dve_stt,
                "dqc": dve_dqc, "mg": dve_mg, "dlt": dve_dlt}
    queues = [pe_q, act_q, dve_q, sp_q]
    idx = [0] * len(queues)
    while any(i < len(q) for i, q in zip(idx, queues)):
        progressed = False
        for qi, q in enumerate(queues):
            while idx[qi] < len(q) and deps_ready(q[idx[qi]]):
                kind, a = q[idx[qi]]
                emitters[kind](a)
                idx[qi] += 1
                progressed = True
        if not progressed:
            raise RuntimeError("plan deadlock")

    # ---------------- batched softmax-combine tail ----------------
    # negMq[:, k] = M_{b,q} (min over chunk of csum_ps = -max csum)
    # negMg = min_q -> M_b; cq = exp(M_q - M_b)
    # pred = (sum_q cq*numer_q) / (sum_q cq*denom_q)
    e_cq = P.step("ACT", lambda: nc.scalar.activation(
        out=cq_all[:], in_=cqd[:], func=AF.Exp, bias=zcol[:],
        scale=-1.0), [("dve", t_dlt[BS - 1])])
    P.step("DVE", lambda: nc.vector.tensor_mul(
        out=prod[:], in0=cq_all[:], in1=dq_all[:]),
        [("act", e_cq)])
    P.step("DVE", lambda: nc.vector.tensor_reduce(
        out=Dall[:], in_=prod[:].rearrange("p (b q) -> p b q", q=NQ),
        axis=AX, op=OP.add), [])
    P.step("DVE", lambda: nc.vector.reciprocal(out=rDall[:], in_=Dall[:]), [])
    P.step("DVE", lambda: nc.vector.tensor_mul(
        out=prod[:], in0=cq_all[:], in1=nq_all[:]), [])
    P.step("DVE", lambda: nc.vector.tensor_reduce(
        out=Sraw[:], in_=prod[:].rearrange("p (b q) -> p b q", q=NQ),
        axis=AX, op=OP.add), [])
    P.step("DVE", lambda: nc.vector.tensor_mul(
        out=preds[:], in0=Sraw[:], in1=rDall[:]), [])

    # ---------------- tail: store output ----------------
    # row 0 of preds is already the prior-mean pred0 (uniform weights).
    # out_s is [N, BS]; the host transposes (pure marshalling).
    P.step("SP", lambda: nc.sync.dma_start(out=out_h[:], in_=preds[:]),
           [("dve", P.counts["dve"])], inc=("dout", 16))
    P.step("SP", None, [("dout", 16)], inc=False)

    # ---------------- emit ----------------
    with ctx:
        sems = {}
        for name in ("pe", "act", "dve", "pool", "dout", "dl", "dwt", "dxn",
                     "dtr", "dx1", "dx2", "dx3",
                     "dr0", "dr0b", "dr1", "dr2"):
            sems[name] = ctx.enter_context(nc.semaphore(name=f"sem_{name}"))

        def run(eng_name, engine):
            for emit, waits, inc, fuse in P.steps[eng_name]:
                if eng_name in ("SP", "POOL", "DVE"):
                    fuse = False
                if fuse and emit is not None and waits:
                    # fuse ONE wait into the instruction (1 wait slot per
                    # instruction): saves the standalone EventSemaphore
                    # decode+dispatch (~57ns) on the issuing engine
                    for sem_name, thr in waits[:-1]:
                        engine.wait_ge(sems[sem_name], thr)
                    inst = emit()
                    sem_name, thr = waits[-1]
                    try:
                        inst.wait_op(sems[sem_name], thr, "sem-ge")
                    except Exception:
                        raise RuntimeError(
                            f"wait fuse failed on {eng_name}")
                else:
                    for sem_name, thr in waits:
                        engine.wait_ge(sems[sem_name], thr)
                    inst = emit() if emit is not None else None
                if inst is not None and inc is not None:
                    inst.then_inc(sems[inc[0]], inc[1])

        with nc.Block() as block:
            @block.sync
            def _(eng):
                run("SP", eng)

            @block.gpsimd
            def _(eng):
                run("POOL", eng)

            @block.tensor
            def _(eng):
                run("PE", eng)

            @block.scalar
            def _(eng):
                run("ACT", eng)

            @block.vector
            def _(eng):
                run("DVE", eng)

    return nc


_NC = None


def _get_nc():
    global _NC
    if _NC is None:
        _NC = build_nc()
    return _NC


def _f32r_round(a):
    # emulate the gpsimd casting DMA's f32->f32r rounding (host-side), so
    # the dram tensors can be plain (pre-rounded) f32r and the startup DMAs
    # non-casting. keep/mode via env for calibration.
    keep = int(os.environ.get("F32R_KEEP", "10"))
    mode = os.environ.get("F32R_MODE", "rn")
    b = np.ascontiguousarray(a, dtype=np.float32).view(np.uint32)
    sh = np.uint32(23 - keep)
    mask = np.uint32(0xFFFFFFFF) << sh
    if mode == "rn":
        half = (np.uint32(1) << np.uint32(22 - keep))
        b = (b + half) & mask
    else:
        b = b & mask
    return b.view(np.float32)


def kernel(data, targets, W, _trace=False, _tc=None):
    data = np.ascontiguousarray(np.asarray(data), dtype=np.float32)
    targets = np.ascontiguousarray(np.asarray(targets), dtype=np.float32)
    W = np.ascontiguousarray(np.asarray(W), dtype=np.float32)
    nc = _get_nc()
    # host-side operand packing (pure data marshalling; all compute,
    # including the y - x.w fusion, runs on device)
    rhs_p = np.concatenate([-W, np.ones((1, T), np.float32)], axis=0)
    lmat_p = np.ascontiguousarray(
        (0.5 * np.tril(np.ones((N, N), np.float32), -1).T))
    # [W^T | 1] f16 tiles: wt16_p[t_local, g*(D+1)+c]
    wtt = np.concatenate([W.T.astype(np.float16),
                          np.ones((T, 1), np.float16)], axis=1)  # [T, 65]
    wt16_p = np.ascontiguousarray(
        wtt.reshape(NG, 128, D + 1).transpose(1, 0, 2).reshape(
            128, NG * (D + 1)))
    in_maps = []
    for c in range(NCORES):
        sl = slice(c * BS, (c + 1) * BS)
        dT = data[sl].transpose(2, 0, 1).reshape(D, BS * N)    # d, (b n)
        yrow = targets[sl].reshape(1, BS * N)
        lhsT_p = np.concatenate([dT, yrow], axis=0)
        ops_p = np.ascontiguousarray(np.concatenate(
            [lhsT_p[:, 0:N], rhs_p, lhsT_p[:, N:]], axis=1))
        xn16_p = np.ascontiguousarray(
            data[sl].transpose(1, 0, 2).reshape(N, BS * D).astype(np.float16))
        in_maps.append({
            "ops_p": ops_p,
            "lmat_p": lmat_p,
            "wt16_p": wt16_p,
            "xn16_p": xn16_p,
        })
    kw = {}
    if _trace:
        kw = dict(trace=True, trace_cores=_tc if _tc is not None else [0])
    res = run_bass_kernel_spmd(nc, in_maps, core_ids=list(range(NCORES)), **kw)
    out = np.concatenate([r["out_s"].T for r in res.results], axis=0)
    if _trace:
        return out, res
    return out


if __name__ == "__main__":
    rng = np.random.default_rng(0)
    data = rng.standard_normal((B, N, D), dtype=np.float32)
    targets = rng.standard_normal((B, N), dtype=np.float32)
    W = rng.standard_normal((D, T), dtype=np.float32)
    out = kernel(data, targets, W)
    print("out", out.shape, out.dtype, np.abs(out).mean())


# revision 40
# speedup vs baseline: 1.0258x; 1.0258x over previous
"""Trainium2 Bass kernel for nn_DiscreteMMSE — f32r pipeline, PE-side numerator.

Reference computation (per batch b):
    proj[n,t] = data[b,n,:] @ W[:,t]
    logp      = -0.5*(targets - proj)^2 + const
    csum      = cumsum_n(logp);  alpha = softmax_t(csum[n-1])
    pred[n]   = sum_t alpha[n-1,t] * proj[n,t]   (n>=1)
    pred[0]   = data[b,0,:] @ W.mean(axis=1)

Design (168006ns baseline -> 149136ns; DVE ran min+stt at 2384ns/chunk):
  * err[n,t] = y - x.w via ONE augmented f32r matmul (lhsT=[data^T;y],
    rhs=[-W;1]) -> err_ps. ACT squares it (err2, f32r SBUF); PE csum via
    POSITIVE 0.5*L_strict matmul -> csum_ps; DVE min -> ACT exp(scale=-1,
    bias=-min) -> expw f16. Row 0 degenerates to uniform weights == the
    reference's prior-mean pred0 (no special case).
  * NEW numerator path: instead of recomputing err (late matmul) and a
    full-width DVE scalar_tensor_tensor (1192ns), note
        sum_t expw*proj = x . (W_chunk @ expw^T)      (w~ = W @ expw^T)
        sum_t expw      = ones @ expw^T               (denominator)
    One SBUF->SBUF dma_start_transpose per chunk turns expw [128,1024]
    into 8 stacked [128t,128n] f16 tiles (idle DMA hw, issued from idle
    SP). PE contracts them with host-packed [W^T|1] f16 tiles into
    wt_ps[n,65] (8 x 65-col f16 matmuls, ~520 PE cycles). DVE then does a
    64-el stt (x16 . wt) for the numerator and a 1-el copy for the
    denominator column. DVE/chunk: 2384 -> ~1650ns. ACT drops the exp
    accum_out (denominator now free from the ones row): 2263 -> ~2080ns,
    the new steady-state bottleneck (91%+ busy). PE: ~2570 cols/chunk,
    fully ramped to 2.4GHz via startup warmup dummies.
  * PSUM: err 1 slot (2 banks) + csum 3 slots (6 banks) = all 8 banks;
    the wt accumulators alias the first 65 cols of csum slot (k+2)%3,
    dead between exp(k+2) and csum(k+5).
  * ACT runs exp(k) TWO cycles after min(k) (act order [exp(k), sq(k+3)])
    so the sq->csum->min->exp chain (~2.5us) never stalls ACT; the wt
    matmuls trail by WLAG=4 chunks to hide the ~3.1us transpose-DMA chain.
  * per-batch negMg + delta on idle DVE mid-loop; ONE 64-col exp at the
    end replaces 16 per-batch cq activations.
  * semaphore waits on the hot ops (err/sq/min/exp) are FUSED into the
    instructions (wait_op; 1 wait slot per instr, rest standalone),
    removing the ~45-57ns standalone EventSemaphore decode from the
    csum->min->exp chain. ACT engine ends up 95% busy; steady state is
    the ACT floor sq+exp = 2x1038ns/chunk.
  * tail: cq = exp(M_q - M_b);
    pred = (sum_q cq*numer_q) / (sum_q cq*denom_q)  -- no targets needed.

Raw bass with explicit semaphores via a small planner (one wait_ge per
dependency, elided when implied by program order or earlier waits).

Sharded batch-parallel over 8 cores: 16 batches/core, W replicated.
kernel() host-packs ops/W^T/x16 per core (pure data marshalling; all
compute runs on device).
"""

from contextlib import ExitStack

import numpy as np

import concourse.bass as bass
from concourse import mybir
from concourse.bass_utils import run_bass_kernel_spmd

B, N, D, T = 128, 128, 64, 4096
NCORES = 8
BS = B // NCORES          # batches per core
CW = 1024                 # task-axis chunk width (2 PSUM banks fp32)
NQ = T // CW              # chunks per batch
MM = 512                  # one PSUM bank of fp32
NK = BS * NQ              # total chunk count
NH = NK * 2               # total half (512) count
NT = CW // 128            # 128-wide t-tiles per chunk (transpose grain)
NG = T // 128             # global t-tile count (W^T tiles)

F32 = mybir.dt.float32
F32R = mybir.dt.float32r
F16 = mybir.dt.float16
AX = mybir.AxisListType.X
OP = mybir.AluOpType
AF = mybir.ActivationFunctionType

import os
NCSUM = 3                 # csum PSUM slots (3 x 2 banks)
SELF_WAITS = os.environ.get("SELF_WAITS", "1") == "1"
WLAG = int(os.environ.get("WLAG", "4"))    # wt lag behind exp-chunk index
NE2 = 3                   # err2 chunk buffers
NEXPW = 5                 # expw chunk buffers
NTR = 4                   # expwT chunk buffers
NSCR = 4                  # stt dummy-output rotation (avoid DVE WAW overlap)
SELF_LAG = int(os.environ.get("SELF_LAG", "2"))


class Planner:
    """Per-engine step lists with resolved single-sem wait thresholds."""

    def __init__(self):
        self.steps = {"PE": [], "ACT": [], "DVE": [], "POOL": [], "SP": []}
        self.counts = {}
        self.waited = {e: {} for e in self.steps}

    self_waits = True
    self_lag = 1

    def step(self, eng, emit, waits=(), inc=None, fuse=False):
        waits = list(waits)
        if self.self_waits and eng in ("DVE", "POOL"):
            prev = self.counts.get(eng.lower(), 0) - (self.self_lag - 1)
            if prev > 0:
                waits.insert(0, (eng.lower(), prev))
        real = []
        for sem_name, thr in waits:
            if thr is None or thr <= 0:
                continue
            if self.waited[eng].get(sem_name, 0) >= thr:
                continue
            self.waited[eng][sem_name] = thr
            real.append((sem_name, thr))
        if inc is None:
            inc = (eng.lower(), 1)
        if inc is not False:
            self.counts.setdefault(inc[0], 0)
            self.counts[inc[0]] += inc[1]
        self.steps[eng].append(
            (emit, real, inc if inc is not False else None, fuse))
        return self.counts[inc[0]] if inc is not False else None


def build_nc():
    nc = bass.Bass("TRN2")
    ctx = ExitStack()
    ctx.enter_context(nc.allow_low_precision(reason="f32r/f16 pipeline"))

    # host-packed operands, laid out [lhsT_b0(128) | rhs(4096) | lhsT_rest]
    # so the first DMA (1152 cols) unblocks batch-0/chunk-0 work asap.
    OPS_W = BS * N + T
    ops_h = nc.dram_tensor("ops_p", [D + 1, OPS_W], F32,
                           kind="ExternalInput")
    wt16_h = nc.dram_tensor("wt16_p", [128, NG * (D + 1)], F16,
                            kind="ExternalInput")
    xn16_h = nc.dram_tensor("xn16_p", [N, BS * D], F16, kind="ExternalInput")
    out_h = nc.dram_tensor("out_s", [N, BS], F32, kind="ExternalOutput")
    # POSITIVE 0.5*strict-lower (transposed): csum_ps = -csum; the row-0
    # column of L is all-zero, so row 0 of the pipeline degenerates to
    # uniform weights == the reference's prior-mean pred0. No special case.
    l_h = nc.dram_tensor("lmat_p", [N, N], F32, kind="ExternalInput")

    def sb(name, shape, dt):
        return ctx.enter_context(nc.sbuf_tensor(name, shape, dt))

    def ps(name, shape, dt):
        return ctx.enter_context(nc.psum_tensor(name, shape, dt))

    l_sb = sb("l_sb", [N, N], F32R)
    ops = sb("ops", [D + 1, OPS_W], F32R)
    rhs = ops[:, N:N + T]                      # [-W ; ones]
    l_r = l_sb[:]

    def lhs_view(b):
        # batch 0 lives at cols 0:128; batches 1.. after rhs
        c0 = 0 if b == 0 else N + T + (b - 1) * N
        return ops[:, c0:c0 + N]
    wt16 = sb("wt16", [128, NG, D + 1], F16)   # [W^T | 1] t-tiles
    xn16 = sb("xn16", [N, BS, D], F16)         # data in [n, d] layout
    preds = sb("preds", [N, BS], F32)
    err2 = [sb(f"err2_{i}", [N, CW], F32R) for i in range(NE2)]
    expw = [sb(f"expw_{i}", [N, CW], F16) for i in range(NEXPW)]
    expwT = [sb(f"expwT_{i}", [128, NT, 128], F16) for i in range(NTR)]
    scr = [sb(f"scr_{i}", [N, D], F32) for i in range(NSCR)]
    negMq = sb("negMq", [N, NK], F32)
    dq_all = sb("dq_all", [N, NK], F32)
    nq_all = sb("nq_all", [N, NK], F32)
    cq_all = sb("cq_all", [N, NK], F32)
    cqd = sb("cqd", [N, NK], F32)
    prod = sb("prod", [N, NK], F32)
    negMg_t = sb("negMg_t", [N, BS], F32)
    Dall = sb("Dall", [N, BS], F32)
    rDall = sb("rDall", [N, BS], F32)
    Sraw = sb("Sraw", [N, BS], F32)

    # PSUM: err 1 slot (2 banks) + csum 3 slots (6 banks) = full 16KB.
    # ACT runs exp(k) TWO cycles after min(k) (act cycle = [exp(k),
    # sq(k+3)]), so the sq->csum->min->exp chain (~2.5us) spans two act
    # cycles and ACT never stalls on it. The wt accumulators alias the
    # first 65 cols of csum slot (k+2)%3, which is dead from exp(k+2)
    # until csum(k+5).
    err_ps = ps("err_ps", [N, CW], F32)
    csum_ps = [ps(f"csum_ps_{i}", [N, CW], F32) for i in range(NCSUM)]
    wt_view = [csum_ps[i][:, 0:D + 1] for i in range(NCSUM)]

    zcol = sb("zcol", [128, 1], F32)
    dum = sb("dum", [128, 256], F32R)

    P = Planner()
    P.self_waits = SELF_WAITS
    P.self_lag = SELF_LAG

    # ---------------- DMAs ----------------
    # all operands are f32/f16 (f32r is a bitcast view), so no casting
    # DMAs are needed: the critical startup pieces ride SP/HWDGE (fast
    # issue), the rest go on the pool/SWDGE queue in parallel.
    def dma_ops(eng, c0, c1):
        return lambda: eng.dma_start(out=ops[:, c0:c1],
                                     in_=ops_h[:, c0:c1])

    pool_dmas = [
        ("dr0", dma_ops(nc.gpsimd, 0, N + CW)),    # lhsT_b0 + rhs chunk 0
        ("dl", lambda: nc.gpsimd.dma_start(out=l_sb[:], in_=l_h[:])),
        ("dr1", dma_ops(nc.gpsimd, N + CW, N + 2 * CW)),
        ("dr2", dma_ops(nc.gpsimd, N + 2 * CW, N + 4 * CW)),
        ("dx1", dma_ops(nc.gpsimd, N + T, N + T + 5 * N)),  # lhsT b1-5
        ("dx2", dma_ops(nc.gpsimd, N + T + 5 * N, N + T + 10 * N)),
        ("dx3", dma_ops(nc.gpsimd, N + T + 10 * N, N + T + 15 * N)),
    ]
    for s, d in pool_dmas:
        P.step("POOL", d, inc=(s, 16))
    # f16 operands are non-casting; issue from the idle SP/HWDGE queue
    sp_dmas = [
        ("dwt", lambda: nc.sync.dma_start(
            out=wt16[:], in_=wt16_h[:].rearrange(
                "p (g c) -> p g c", c=D + 1))),
        ("dxn", lambda: nc.sync.dma_start(
            out=xn16[:], in_=xn16_h[:].rearrange(
                "p (b d) -> p b d", d=D))),
    ]
    for s, d in sp_dmas:
        P.step("SP", d, inc=(s, 16))
    e_zcol = P.step("DVE", lambda: nc.vector.memset(zcol[:], 0.0))

    # ---------------- main loop ----------------
    # chunk k in [0, NK): 1024 tasks. err pair -> sq -> csum -> min ->
    # exp -> dma-transpose -> wt matmuls -> stt/dqc. Steady state
    # (ACT-bound ~2080ns):
    #   PE  [csum(k), err pair(k+2), wt(k-WLAG)x8]   ~2570 cols
    #   ACT [sq(k+1), exp(k)]                        ~2080
    #   DVE [min(k), stt(k-WLAG), dqc(k-WLAG)]       ~1650
    #   SP  [tr(k)]; DMA 64 xbar tiles/chunk
    t_err, t_sq, t_csum, t_min, t_exp = {}, {}, {}, {}, {}
    t_tr, t_wt, t_stt, t_dqc = {}, {}, {}, {}

    def pe_err(j):
        k = j // 2
        b, q = divmod(k, NQ)
        c0 = q * CW + (j % 2) * MM
        dst = err_ps[:, (j % 2) * MM:(j % 2) * MM + MM]
        rq = "dr0" if q == 0 else ("dr1" if q == 1 else "dr2")
        xq = "dr0" if b == 0 else f"dx{min(3, 1 + (b - 1) // 5)}"
        w = [(xq, 16), (rq, 16)]
        if k >= 1:
            w.append(("act", t_sq[k - 1]))     # slot reused after sq(k-1)
        t_err[j] = P.step("PE", (lambda b=b, c0=c0, dst=dst: nc.tensor.matmul(
            dst, lhs_view(b), rhs[:, c0:c0 + MM],
            start=True, stop=True)), w, fuse=True)

    def act_sq(k):
        dst = err2[k % NE2]
        w = [("pe", t_err[2 * k + 1]), ("dve", e_zcol)]
        kf = k - NE2
        if kf >= 0:
            w.append(("pe", t_csum[kf]))
        t_sq[k] = P.step("ACT", (lambda dst=dst:
                                 nc.scalar.activation(
            out=dst[:], in_=err_ps[:], func=AF.Square, bias=zcol[:],
            scale=1.0)), w, fuse=True)

    def pe_csum(k):
        s = k % NCSUM
        w = [("act", t_sq[k])]
        if k < 2:
            w.append(("dl", 16))
        kf = k - NCSUM
        if kf >= 0:
            w.append(("act", t_exp[kf]))       # full-slot reuse
        ka = k - WLAG - 1                      # wt(ka) aliased cols 0:65
        if ka >= 0:
            w.append(("dve", t_dqc[ka]))

        def emit(s=s, k=k):
            e2 = err2[k % NE2]
            nc.tensor.matmul(csum_ps[s][:, 0:MM], l_r, e2[:, 0:MM],
                             start=True, stop=True)
            return nc.tensor.matmul(csum_ps[s][:, MM:2 * MM], l_r,
                                    e2[:, MM:2 * MM], start=True, stop=True)
        t_csum[k] = P.step("PE", emit, w, fuse=False)

    def dve_min(k):
        s = k % NCSUM
        t_min[k] = P.step("DVE", (lambda k=k, s=s: nc.vector.tensor_reduce(
            out=negMq[:, k:k + 1], in_=csum_ps[s][:], axis=AX, op=OP.min)),
            [("pe", t_csum[k])], fuse=True)

    def act_exp(k):
        s = k % NCSUM
        # expw-buffer reuse (tr(k-NEXPW) done) is implied: exp waits
        # min(k) <- csum(k) <- PE order after wt(k-WLAG-1) whose first
        # matmul waited dtr >= 16*(k-WLAG) = 16*(k-NEXPW+1).
        w = [("dve", t_min[k])]
        t_exp[k] = P.step("ACT", (lambda k=k, s=s: nc.scalar.activation(
            out=expw[k % NEXPW][:], in_=csum_ps[s][:], func=AF.Exp,
            bias=negMq[:, k:k + 1], scale=-1.0)), w, fuse=True)

    def sp_tr(k):
        w = [("act", t_exp[k])]
        kf = k - NTR
        if kf >= 0:
            w.append(("pe", t_wt[kf]))         # expwT slot reused after wt
        t_tr[k] = P.step("SP", (lambda k=k: nc.sync.dma_start_transpose(
            out=expwT[k % NTR][:], in_=expw[k % NEXPW][:])),
            w, inc=("dtr", 16))

    def pe_wt(k):
        s = (k + 2) % NCSUM
        b, q = divmod(k, NQ)
        ke = min(k + 2, NK - 1)                # alias slot dead after exp
        w = [("dtr", 16 * (k + 1)), ("dwt", 16), ("act", t_exp[ke])]
        if k >= NCSUM:
            w.append(("dve", t_dqc[k - NCSUM]))  # prior alias use drained

        def emit(s=s, k=k, q=q):
            last = None
            for g in range(NT):
                last = nc.tensor.matmul(
                    wt_view[s], expwT[k % NTR][:, g, :],
                    wt16[:, NT * q + g, :],
                    start=(g == 0), stop=(g == NT - 1))
            return last
        t_wt[k] = P.step("PE", emit, w, fuse=False)

    def dve_stt(k):
        s = (k + 2) % NCSUM
        b = k // NQ
        t_stt[k] = P.step("DVE", (lambda k=k, s=s, b=b:
                                  nc.vector.scalar_tensor_tensor(
            out=scr[k % NSCR][:], in0=wt_view[s][:, 0:D], scalar=1.0,
            in1=xn16[:, b, :], op0=OP.mult, op1=OP.mult,
            accum_out=nq_all[:, k:k + 1])),
            [("pe", t_wt[k]), ("dxn", 16)])

    def dve_dqc(k):
        s = (k + 2) % NCSUM
        t_dqc[k] = P.step("DVE", (lambda k=k, s=s: nc.vector.tensor_scalar(
            out=dq_all[:, k:k + 1], in0=wt_view[s][:, D:D + 1], scalar1=1.0,
            scalar2=None, op0=OP.mult)),
            [("pe", t_wt[k])])

    # PE p-state warmup: ~14 x 256-col f32r dummy matmuls (~5.5us of PE
    # busy from t~0.4us) keep the ramp alive until the first real err
    # matmuls, which then run at full 2.4GHz instead of 0.65/1.2.
    for _ in range(14):
        P.step("PE", lambda: nc.tensor.matmul(
            err_ps[:, 0:256], dum[:, 0:128], dum[:],
            start=True, stop=True), [], inc=False)

    # emission order per engine (greedy against emission-time deps)
    pe_q = [("err", 0), ("err", 1)]
    for k in range(NK):
        if 2 * k + 2 < NH:
            pe_q.append(("err", 2 * k + 2))
            pe_q.append(("err", 2 * k + 3))
        pe_q.append(("csum", k))
        if k - WLAG >= 0:
            pe_q.append(("wt", k - WLAG))
    for k in range(NK - WLAG, NK):
        pe_q.append(("wt", k))

    t_mg = {}
    t_dlt = {}

    def dve_mg(b):
        # per-batch negMg on idle DVE mid-loop; placed >=2 DVE ops after
        # min(4b+3) so the lag-2 self-wait proves it complete.
        t_mg[b] = P.step("DVE", (lambda b=b: nc.vector.tensor_reduce(
            out=negMg_t[:, b:b + 1],
            in_=negMq[:, b * NQ:(b + 1) * NQ], axis=AX, op=OP.min)), [])

    def dve_dlt(b):
        # cqd = negMq - negMg (broadcast); >=2 DVE ops after mg(b)
        t_dlt[b] = P.step("DVE", (lambda b=b: nc.vector.tensor_scalar(
            out=cqd[:, b * NQ:(b + 1) * NQ],
            in0=negMq[:, b * NQ:(b + 1) * NQ],
            scalar1=negMg_t[:, b:b + 1], scalar2=None,
            op0=OP.subtract)), [])

    # ACT cycle = [exp(k), sq(k+3)]: exp trails min(k) by two cycles, so
    # the sq->csum->min chain never stalls ACT.
    act_q = [("sq", 0), ("sq", 1), ("sq", 2)]
    dve_q = []
    sp_q = []
    for k in range(NK):
        act_q.append(("exp", k))
        if k + 3 < NK:
            act_q.append(("sq", k + 3))
        sp_q.append(("tr", k))
        dve_q.append(("min", k))
        if k - WLAG >= 0:
            dve_q.append(("stt", k - WLAG))
        if k - WLAG >= 0:
            dve_q.append(("dqc", k - WLAG))
        # lag-2 self-waits imply: at issue of op p, all ops <= p-2 are
        # complete. mg(b) sits >=2 after min(4b+3); dlt(b) one cycle later,
        # >=2 after mg(b). Placed last so dqc (alias chain) isn't delayed.
        if k % NQ == 0 and k > 0:
            dve_q.append(("mg", k // NQ - 1))
        if k % NQ == 1 and k > NQ:
            dve_q.append(("dlt", k // NQ - 1))
    flush = []
    for k in range(NK - WLAG, NK):
        flush.append(("stt", k))
        flush.append(("dqc", k))
    flush.insert(2, ("mg", BS - 1))
    flush.insert(5, ("dlt", BS - 1))
    dve_q.extend(flush)

    def deps_ready(item):
        kind, a = item
        if kind == "err":
            return a // 2 < 1 or (a // 2 - 1) in t_sq
        if kind == "sq":
            if (2 * a + 1) not in t_err:
                return False
            kf = a - NE2
            return kf < 0 or kf in t_csum
        if kind == "csum":
            if a not in t_sq:
                return False
            kf = a - NCSUM
            if kf >= 0 and kf not in t_exp:
                return False
            ka = a - WLAG - 1
            return ka < 0 or ka in t_dqc
        if kind == "min":
            return a in t_csum
        if kind == "exp":
            if a not in t_min:
                return False
            kf = a - NEXPW
            return kf < 0 or kf in t_tr
        if kind == "tr":
            if a not in t_exp:
                return False
            kf = a - NTR
            return kf < 0 or kf in t_wt
        if kind == "wt":
            if a not in t_tr:
                return False
            if min(a + 2, NK - 1) not in t_exp:
                return False
            kf = a - NCSUM
            return kf < 0 or kf in t_dqc
        if kind == "stt":
            return a in t_wt
        if kind == "dqc":
            return a in t_wt and a in t_stt
        if kind == "mg":
            return (a * NQ + 3) in t_min
        if kind == "dlt":
            return a in t_mg
        raise ValueError(kind)

    emitters = {"err": pe_err, "sq": act_sq, "csum": pe_csum, "min": dve_min,
                "exp": act_exp, "tr": sp_tr, "wt": pe_wt, "stt": dve_stt,
                "dqc": dve_dqc, "mg": dve_mg, "dlt": dve_dlt}
    queues = [pe_q, act_q, dve_q, sp_q]
    idx = [0] * len(queues)
    while any(i < len(q) for i, q in zip(idx, queues)):
        progressed = False
        for qi, q in enumerate(queues):
            while idx[qi] < len(q) and deps_ready(q[idx[qi]]):
                kind, a = q[idx[qi]]
                emitters[kind](a)
                idx[qi] += 1
                progressed = True
        if not progressed:
            raise RuntimeError("plan deadlock")

    # ---------------- batched softmax-combine tail ----------------
    # negMq[:, k] = M_{b,q} (min over chunk of csum_ps = -max csum)
    # negMg = min_q -> M_b; cq = exp(M_q - M_b)
    # pred = (sum_q cq*numer_q) / (sum_q cq*denom_q)
    e_cq = P.step("ACT", lambda: nc.scalar.activation(
        out=cq_all[:], in_=cqd[:], func=AF.Exp, bias=zcol[:],
        scale=-1.0), [("dve", t_dlt[BS - 1])])
    P.step("DVE", lambda: nc.vector.tensor_mul(
        out=prod[:], in0=cq_all[:], in1=dq_all[:]),
        [("act", e_cq)])
    P.step("DVE", lambda: nc.vector.tensor_reduce(
        out=Dall[:], in_=prod[:].rearrange("p (b q) -> p b q", q=NQ),
        axis=AX, op=OP.add), [])
    P.step("DVE", lambda: nc.vector.reciprocal(out=rDall[:], in_=Dall[:]), [])
    P.step("DVE", lambda: nc.vector.tensor_mul(
        out=prod[:], in0=cq_all[:], in1=nq_all[:]), [])
    P.step("DVE", lambda: nc.vector.tensor_reduce(
        out=Sraw[:], in_=prod[:].rearrange("p (b q) -> p b q", q=NQ),
        axis=AX, op=OP.add), [])
    P.step("DVE", lambda: nc.vector.tensor_mul(
        out=preds[:], in0=Sraw[:], in1=rDall[:]), [])

    # ---------------- tail: store output ----------------
    # row 0 of preds is already the prior-mean pred0 (uniform weights).
    # out_s is [N, BS]; the host transposes (pure marshalling).
    P.step("SP", lambda: nc.sync.dma_start(out=out_h[:], in_=preds[:]),
           [("dve", P.counts["dve"])], inc=("dout", 16))
    P.step("SP", None, [("dout", 16)], inc=False)

    # ---------------- emit ----------------
    with ctx:
        sems = {}
        for name in ("pe", "act", "dve", "pool", "dout", "dl", "dwt", "dxn",
                     "dtr", "dx1", "dx2", "dx3",
                     "dr0", "dr0b", "dr1", "dr2"):
            sems[name] = ctx.enter_context(nc.semaphore(name=f"sem_{name}"))

        def run(eng_name, engine):
            for emit, waits, inc, fuse in P.steps[eng_name]:

                if fuse and emit is not None and waits:
                    # fuse ONE wait into the instruction (1 wait slot per
                    # instruction): saves the standalone EventSemaphore
                    # decode+dispatch (~57ns) on the issuing engine
                    for sem_name, thr in waits[:-1]:
                        engine.wait_ge(sems[sem_name], thr)
                    inst = emit()
                    sem_name, thr = waits[-1]
                    try:
                        inst.wait_op(sems[sem_name], thr, "sem-ge")
                    except Exception:
                        raise RuntimeError(
                            f"wait fuse failed on {eng_name}")
                else:
                    for sem_name, thr in waits:
                        engine.wait_ge(sems[sem_name], thr)
                    inst = emit() if emit is not None else None
                if inst is not None and inc is not None:
                    inst.then_inc(sems[inc[0]], inc[1])

        with nc.Block() as block:
            @block.sync
            def _(eng):
                run("SP", eng)

            @block.gpsimd
            def _(eng):
                run("POOL", eng)

            @block.tensor
            def _(eng):
                run("PE", eng)

            @block.scalar
            def _(eng):
                run("ACT", eng)

            @block.vector
            def _(eng):
                run("DVE", eng)

    return nc


_NC = None


def _get_nc():
    global _NC
    if _NC is None:
        _NC = build_nc()
    return _NC


def _f32r_round(a):
    # emulate the gpsimd casting DMA's f32->f32r rounding (host-side), so
    # the dram tensors can be plain (pre-rounded) f32r and the startup DMAs
    # non-casting. keep/mode via env for calibration.
    keep = int(os.environ.get("F32R_KEEP", "10"))
    mode = os.environ.get("F32R_MODE", "rn")
    b = np.ascontiguousarray(a, dtype=np.float32).view(np.uint32)
    sh = np.uint32(23 - keep)
    mask = np.uint32(0xFFFFFFFF) << sh
    if mode == "rn":
        half = (np.uint32(1) << np.uint32(22 - keep))
        b = (b + half) & mask
    else:
        b = b & mask
    return b.view(np.float32)


def kernel(data, targets, W, _trace=False, _tc=None):
    data = np.ascontiguousarray(np.asarray(data), dtype=np.float32)
    targets = np.ascontiguousarray(np.asarray(targets), dtype=np.float32)
    W = np.ascontiguousarray(np.asarray(W), dtype=np.float32)
    nc = _get_nc()
    # host-side operand packing (pure data marshalling; all compute,
    # including the y - x.w fusion, runs on device)
    rhs_p = np.concatenate([-W, np.ones((1, T), np.float32)], axis=0)
    lmat_p = np.ascontiguousarray(
        (0.5 * np.tril(np.ones((N, N), np.float32), -1).T))
    # [W^T | 1] f16 tiles: wt16_p[t_local, g*(D+1)+c]
    wtt = np.concatenate([W.T.astype(np.float16),
                          np.ones((T, 1), np.float16)], axis=1)  # [T, 65]
    wt16_p = np.ascontiguousarray(
        wtt.reshape(NG, 128, D + 1).transpose(1, 0, 2).reshape(
            128, NG * (D + 1)))
    in_maps = []
    for c in range(NCORES):
        sl = slice(c * BS, (c + 1) * BS)
        dT = data[sl].transpose(2, 0, 1).reshape(D, BS * N)    # d, (b n)
        yrow = targets[sl].reshape(1, BS * N)
        lhsT_p = np.concatenate([dT, yrow], axis=0)
        ops_p = np.ascontiguousarray(np.concatenate(
            [lhsT_p[:, 0:N], rhs_p, lhsT_p[:, N:]], axis=1))
        xn16_p = np.ascontiguousarray(
            data[sl].transpose(1, 0, 2).reshape(N, BS * D).astype(np.float16))
        in_maps.append({
            "ops_p": ops_p,
            "lmat_p": lmat_p,
            "wt16_p": wt16_p,
            "xn16_p": xn16_p,
        })
    kw = {}
    if _trace:
        kw = dict(trace=True, trace_cores=_tc if _tc is not None else [0])
    res = run_bass_kernel_spmd(nc, in_maps, core_ids=list(range(NCORES)), **kw)
    out = np.concatenate([r["out_s"].T for r in res.results], axis=0)
    if _trace:
        return out, res
    return out


if __name__ == "__main__":
    rng = np.random.default_rng(0)
    data = rng.standard_normal((B, N, D), dtype=np.float32)
    targets = rng.standard_normal((B, N), dtype=np.float32)
    W = rng.standard_normal((D, T), dtype=np.float32)
    out = kernel(data, targets, W)
    print("out", out.shape, out.dtype, np.abs(out).mean())
